# revision 56
# baseline (speedup 1.0000x reference)
"""PVT-style spatial-reduction attention on 8 TRN2 NeuronCores.

Sharding: core c -> (batch b = c//2, head-group g = c%2), 4 heads each.
No collectives: each core computes outT_partial (512, 4096) in bf16;
host sums the two partials per batch and un-permutes tokens.

Structure (165.7us cost-model time; was 176.2us before this pass):
 - Tokens host-permuted to tap-major order (im2col for the stride-2
   conv is a pure permutation), so conv rhs reads are contiguous.
 - Input DMAs ordered along the critical chain (all transfers serialize
   on one DMA device): qwl, xh8 split per tap t (each unblocks a q
   chunk-pair), wch, wcl, xl8 position-half 0 (closes conv0), kvw,
   xl8 half 1 (closes conv1), pw. Params merged (qw+ql, wch+wcl,
   kw+vw, kbr+vb) to cut per-DMA overhead.
 - PE ramp pre-warmed by junk matmuls gated only on an immediate
   memset, so q runs at full p-state from its first instruction.
 - LN rstd broadcast in bf16: the xrn normalize hits the DVE 2x/4x
   all-bf16-SBUF perf mode.
 - Proj drain: after the main loop the scores/av PSUM pools are
   closed and the remaining proj-octs pipeline through a fresh
   4-buffer PSUM pool with alternating ACT/DVE evacuation.
 - Output staged per chunk in [P, 4, CHUNK] tiles and DMA'd per
   oct-PAIR (octs are contiguous out_d row blocks), so the drain's
   second evacuation overlaps the first pair's transfer and HWDGE
   traffic halves.
 - conv and q projection run as fp8e4 DoubleRow matmuls (0.5 c/row,
   2 k-tiles/instr) with hi/lo operand splits for precision; fp8
   tensors are pre-scaled by powers of 2 (SX/SWC/SQW/SK) to stay in
   e4m3's normal range, compensated via ACT scale params.
 - q@k scores: DoubleRow with k_hi in pair-0, k's fp8 storage residual
   k_lo in pair-1, and a stride-0 broadcast feeding q to both pairs,
   so one instr computes (k_hi+k_lo).T @ q at K=64.
 - exp(softmax) split between ACT (true exp) and a custom DVE op
   (EXP8: (quadratic)^8 minimax fit of exp(SCALE/SK*x), ~0.3% err).
 - av is token-partition-major: out(tok,65)=[denom|e@v] via e-as-lhsT
   (no wasted output partitions), normalized by a reciprocal broadcast
   multiply, then PE-transposed back to feature-major for the proj.
   s-outer accumulation order (t-outer corrupts PSUM pending-zero).
 - Schedule: q first (DMA-gated), conv0/ln0/k0, early half-scores for
   the first W tasks interleaved with conv1/k1/v1, then a software-
   pipelined 32-task stream (scores -> av_head -> av_tail -> proj-oct
   spread one per iteration). DMAs are merged into few large transfers
   (each dma_start costs ~625ns of serial descriptor generation).
"""
import sys as _sys
for _p in ("/opt/trn_rl_repo", "/opt/pypackages"):
    if _p not in _sys.path:
        _sys.path.insert(0, _p)

import numpy as np
import ml_dtypes
from contextlib import ExitStack

import concourse.bass as bass
import concourse.mybir as mybir
import concourse.tile as tile
from concourse import bacc
from concourse.bass_utils import run_bass_kernel_spmd
from concourse.masks import make_identity

from concourse.dve_ops import (OPS, CUSTOM_DVE_SPECS, _SUB_OPCODE_FOR_NAME,
                               DveOp)
from concourse.dve_spec import Spec, Src0, C0, C1, C2, sq as dve_sq, lower
from concourse.dve_uop import DveOpSpec
from concourse.dve_table_gen import dve_ver_for

BF = mybir.dt.bfloat16
F32 = mybir.dt.float32
FP8 = mybir.dt.float8e4
NF8 = ml_dtypes.float8_e4m3
NBF = np.dtype(ml_dtypes.bfloat16)
DRM = mybir.MatmulPerfMode.DoubleRow

P = 128
BS, N, DIM, HEADS, HD = 4, 4096, 512, 8, 64
NKV = 1024
SCALE = HD ** -0.5  # 0.125
EPS = 1e-5
CHUNK = 512
NCH = 8
# fp8 pre-scales (powers of 2) keeping values in e4m3's normal range;
# compensated via ACT activation scale params and the exp argument scale.
SX = 4.0     # x
SWC = 32.0   # conv weights
SQW = 32.0   # q weights
SK = 8.0     # k storage (scores come out as 8*q.k)

# ---------------- custom EXP8 DVE op: exp(SCALE*x) ~= q(x)^8 ----------------


def _fit_exp8_coeffs(scale: float, xmax: float = 16.0):
    """q(x) = c0 x^2 + c1 x + c2 with (q(x))^8 ~= exp(scale*x) for raw
    scores x in [-xmax, xmax]."""
    x = np.linspace(-xmax, xmax, 8001)
    t = np.exp(scale * x / 8.0)
    w = 1.0 / t
    A = np.stack([x * x, x, np.ones_like(x)], axis=1) * w[:, None]
    coef, *_ = np.linalg.lstsq(A, t * w, rcond=None)
    return float(coef[0]), float(coef[1]), float(coef[2])


EXP8_C = _fit_exp8_coeffs(SCALE / SK, xmax=14.0 * SK)


def _register_exp8():
    name = "EXP8_ANT"
    if name in _SUB_OPCODE_FOR_NAME:
        return next(op for op in OPS if op.name == name)
    spec = Spec(
        body=dve_sq(dve_sq(dve_sq((Src0 * C0 + C1) * Src0 + C2))),
        reference=lambda in0, in1, s0, s1, imm2: (
            (((in0 * s0 + s1) * in0 + imm2).astype(np.float32) ** 2) ** 2) ** 2,
    )
    row = max(_SUB_OPCODE_FOR_NAME.values()) + 1
    _SUB_OPCODE_FOR_NAME[name] = row
    ver = dve_ver_for("TRN2")
    spec_c = DveOpSpec(name=name, opcode=row, uops=lower(spec, ver=ver),
                       rd1_en=False)
    op = DveOp(name, spec, subdim=False, uops_sha={ver: spec_c.sha(ver)})
    OPS.append(op)
    CUSTOM_DVE_SPECS[name] = spec
    return op


EXP8 = _register_exp8()

EXP_DVE_EXTRA = int(__import__("os").environ.get("K_EXP_DVE_EXTRA", "3"))


D1_EARLY = int(__import__("os").environ.get("K_D1_EARLY", "0"))


D1_PHASE = int(__import__("os").environ.get("K_D1_PHASE", "3"))


def _exp_on_dve(i, j):
    """Which exp instructions run on DVE (custom op) vs ACT."""
    light = ((i % 4 == D1_PHASE) if EXP_DVE_EXTRA == 3
             else i % 4 >= EXP_DVE_EXTRA)
    if D1_EARLY and light:
        # d=1 task: give DVE an EARLY j so its FIFO isn't parked behind a
        # not-yet-computed late score group
        return j == (D1_EARLY - 1)
    if j == 3:
        return True
    return j == 1 and not light


import os as _os
DUMMY_WARM = int(_os.environ.get("K_DUMMY_WARM", "0"))
DUMMY_SC = int(_os.environ.get("K_DUMMY_SC", "0"))
DUMMY_AV = int(_os.environ.get("K_DUMMY_AV", "0"))
AV_TOUTER = int(_os.environ.get("K_AV_TOUTER", "0"))
AV_FIRST = int(_os.environ.get("K_AV_FIRST", "1"))
XRN_DVE = int(_os.environ.get("K_XRN_DVE", "1"))
QCOPY_ACT = int(_os.environ.get("K_QCOPY_ACT", "1"))
Q_ILV = int(_os.environ.get("K_Q_ILV", "1"))
SPS1 = int(_os.environ.get("K_SPS1", "0"))
ESB_BUFS = int(_os.environ.get("K_ESB_BUFS", "16"))
DEBUG = int(_os.environ.get("K_DEBUG", "0"))
NTCOPY_ALT = int(_os.environ.get("K_NTCOPY_ALT", "0"))
SQ_POOL = int(_os.environ.get("K_SQ_POOL", "0"))


def build_nc():
    nc = bacc.Bacc()
    # x fp8 hi: (t, c, P, pos) so one DMA per tap t unblocks q chunk-pair t
    xh8_d = nc.declare_dram_parameter("xh8", (4, 4, P, NKV), FP8, isOutput=False)
    # x fp8 lo: (half, P, c*t, pos-in-half), one DMA per position-half
    xl8_d = nc.declare_dram_parameter("xl8", (2, P, 16, 512), FP8, isOutput=False)
    wc2_d = nc.declare_dram_parameter("wc2", (32, P, DIM), FP8, isOutput=False)
    qwl_d = nc.declare_dram_parameter("qwl", (P, 2, 2, 2, 256), FP8,
                                      isOutput=False)
    kvw_d = nc.declare_dram_parameter("kvw", (8, P, 256), BF, isOutput=False)
    pw_d = nc.declare_dram_parameter("pw", (2, P, DIM), BF, isOutput=False)
    b4_d = nc.declare_dram_parameter("b4", (P, 4), F32, isOutput=False)
    kvb_d = nc.declare_dram_parameter("kvb", (1, 512), BF, isOutput=False)
    out_d = nc.declare_dram_parameter("out", (DIM, N), BF, isOutput=True)
    if DEBUG:
        dbg = {
            "xrn": nc.declare_dram_parameter("dbg_xrn", (P, 4, NKV), F32,
                                             isOutput=True),
            "qT8": nc.declare_dram_parameter("dbg_qT8", (P, 2, N), F32,
                                             isOutput=True),
            "kT8": nc.declare_dram_parameter("dbg_kT8", (P, 2, 2, NKV), F32,
                                             isOutput=True),
            "v4": nc.declare_dram_parameter("dbg_v4", (P, 8, 4, 65), F32,
                                            isOutput=True),
            "netT": nc.declare_dram_parameter("dbg_netT", (P, 2, N), F32,
                                              isOutput=True),
        }

    with tile.TileContext(nc) as tc, ExitStack() as ctx:
        persist = ctx.enter_context(tc.tile_pool(name="persist", bufs=1))

        xh8 = persist.tile([P, 4, 4, NKV], FP8, tag="xh8")
        xl8 = persist.tile([P, 4, 4, NKV], FP8, tag="xl8")
        wc2 = persist.tile([P, 32, DIM], FP8, tag="wc2")
        qwl = persist.tile([P, 2, 2, 2, 256], FP8, tag="qwl")
        kvw = persist.tile([P, 8, 256], BF, tag="kvw")
        kvb = persist.tile([1, 512], BF, tag="kvb")
        ones1 = persist.tile([1, 512], BF, tag="ones1")
        b4 = persist.tile([P, 4], F32, tag="b4")
        pw = persist.tile([P, 2, DIM], BF, tag="pw")
        wch = wc2[:, 0:16, :]
        wcl = wc2[:, 16:32, :]
        qw8 = qwl[:, 0]
        ql8 = qwl[:, 1]
        kw = kvw[:, 0:4, :]
        vw = kvw[:, 4:8, :]
        kbr = kvb[0:1, 0:256]
        vb = kvb[0:1, 256:512]

        xrc = persist.tile([P, 4, NKV], BF, tag="xrc")
        xrn = persist.tile([P, 4, NKV], BF, tag="xrn")
        # partition halves hold even/odd heads: one 128-part copy
        # fills both heads of a pair (copy cost is free-size only)
        qT8 = persist.tile([P, 2, N], FP8, tag="qT8")
        kT8 = persist.tile([P, 2, 2, NKV], FP8, tag="kT8")
        v4 = persist.tile([P, 8, 4, 65], BF, tag="v4")
        netT = persist.tile([P, 2, N], BF, tag="netT")
        stdt = persist.tile([1, NKV], F32, tag="stdt")
        rstd = persist.tile([1, NKV], BF, tag="rstd")
        ident = persist.tile([P, P], BF, tag="ident")
        vbb = persist.tile([P, 256], BF, tag="vbb")
        ones_inv = persist.tile([P, 1], BF, tag="ones_inv")
        eps1 = persist.tile([1, 1], F32, tag="eps1")

        # ---- DMAs, ordered along the critical chain. All transfers
        # serialize on the one DMA device (~0.36ns/B-per-partition), so
        # arrival time == cumulative bytes; HWDGE descriptor-gen pipelines
        # under transfers. Chain: q needs qwl+xh8(t); conv0 needs
        # wch,wcl,xh8,xl8[h0]; k0 needs kvw; conv1 needs xl8[h1].
        nc.sync.dma_start(qwl[:], qwl_d[:])
        xh8_src = xh8_d[:].rearrange("t c p n -> p c t n")
        for t in range(4):
            nc.sync.dma_start(xh8[:, :, t, :], xh8_src[:, :, t, :])
        wc2_src = wc2_d[:].rearrange("k p n -> p k n")
        nc.sync.dma_start(wc2[:, 0:16, :], wc2_src[:, 0:16, :])
        nc.sync.dma_start(wc2[:, 16:32, :], wc2_src[:, 16:32, :])
        nc.sync.dma_start(xl8[:, :, :, 0:512], xl8_d[0])
        nc.sync.dma_start(b4[:], b4_d[:])
        nc.sync.dma_start(kvb[:], kvb_d[:])
        nc.sync.dma_start(kvw[:], kvw_d[:].rearrange("k p n -> p k n"))
        nc.sync.dma_start(xl8[:, :, :, 512:1024], xl8_d[1])
        nc.sync.dma_start(pw[:], pw_d[:].rearrange("k p n -> p k n"))

        # pre-load the one ACT table covering Ln/Exp/Identity/Square so the
        # fixpoint pass doesn't insert per-switch table loads (1.3us each)
        from concourse.hw_specs import get_activation_tables
        _tset = list(get_activation_tables(nc.m.arch))
        nc.scalar.add_instruction(mybir.InstLoadActFuncSet(
            name=nc.get_next_instruction_name(), ins=[], outs=[],
            act_func_set_id=_tset.index("natural_log_exp_and_others")))

        # ---- init constants (Pool = gpsimd, SBUF only)
        # warm operand for the PE ramp pre-warm: first DVE op, ready ~0.4us
        warm = persist.tile([P, 256], BF, tag="warm")
        nc.vector.memset(warm[:], 1.0)
        nc.vector.memset(ones_inv[:], 1.0 / DIM)
        nc.vector.memset(ones1[:], 1.0)
        nc.vector.memset(eps1[:], EPS)
        nc.gpsimd.memset(v4[:], 0.0)
        nc.gpsimd.memset(v4[:, :, :, 0:1], 1.0)
        make_identity(nc, ident[:])
        nc.gpsimd.partition_broadcast(vbb[:], vb)

        # ---- pools
        esb = ctx.enter_context(tc.tile_pool(name="esb", bufs=ESB_BUFS))
        netp = ctx.enter_context(tc.tile_pool(name="netp", bufs=4))
        recp = ctx.enter_context(tc.tile_pool(name="recp", bufs=4))
        osbp = ctx.enter_context(tc.tile_pool(name="osbp", bufs=2))
        rbbp = ctx.enter_context(tc.tile_pool(name="rbbp", bufs=2))
        sqp = ctx.enter_context(tc.tile_pool(name="sqp", bufs=3))

        dmyps = ctx.enter_context(tc.tile_pool(name="dmy", bufs=1, space="PSUM"))
        ph1 = ExitStack()
        wkps = ph1.enter_context(tc.tile_pool(name="work", bufs=3, space="PSUM"))
        vtps = ph1.enter_context(tc.tile_pool(name="vt", bufs=2, space="PSUM"))
        vpps = ph1.enter_context(tc.tile_pool(name="vp", bufs=1, space="PSUM"))
        esps = ph1.enter_context(tc.tile_pool(name="esps", bufs=2, space="PSUM"))
        eesb = ctx.enter_context(tc.tile_pool(name="eesb", bufs=4 * int(_os.environ.get("K_W", "7")) + 2))

        # ---------------- phase 1 emitters ----------------
        def emit_q(ci, m):
            """q projection for token chunk ci (tap t, half), head pair m."""
            t, half = ci // 2, ci % 2
            cs = slice(ci * CHUNK, (ci + 1) * CHUNK)
            hs = slice(half * 512, (half + 1) * 512)
            qps = wkps.tile([P, CHUNK], F32, tag="wk")
            for u in range(2):
                for wi, wq in enumerate((qw8, ql8)):
                    nc.tensor.matmul(
                        qps[:], wq[:, u, :, m * P:(m + 1) * P],
                        xh8[:, 2 * u:2 * u + 2, t, hs],
                        start=(u == 0 and wi == 0),
                        stop=(u == 1 and wi == 1), perf_mode=DRM)
            if QCOPY_ACT and (ci + m) % 2 == 0:
                nc.scalar.activation(qT8[:, m, cs], qps[:],
                                     mybir.ActivationFunctionType.Identity,
                                     scale=1.0 / (SX * SQW))
            else:
                nc.vector.tensor_scalar_mul(qT8[:, m, cs], qps[:],
                                            1.0 / (SX * SQW))

        def emit_conv(n, after_oct=None):
            """conv for kv positions [n*512, (n+1)*512)."""
            ns = slice(n * 512, (n + 1) * 512)
            vt = vtps.tile([1, 512], F32, tag="vt")
            terms = [(wch, xh8), (wcl, xh8), (wch, xl8)]
            for oct_ in range(4):
                if after_oct is not None:
                    after_oct(oct_)
                cps = wkps.tile([P, 512], F32, tag="wk")
                first = True
                for wt, xt_ in terms:
                    for c in range(4):
                        for v in range(2):
                            w = c * 4 + 2 * v
                            nc.tensor.matmul(
                                cps[:],
                                wt[:, w:w + 2, oct_ * P:(oct_ + 1) * P],
                                xt_[:, c, 2 * v:2 * v + 2, ns],
                                start=first,
                                stop=(xt_ is xl8 and c == 3 and v == 1),
                                perf_mode=DRM)
                            first = False
                # xrc copy on DVE: keeps ACT free for the sq -> var -> LN
                # critical chain (DVE is otherwise idle in this window)
                nc.vector.tensor_scalar(
                    xrc[:, oct_, ns], cps[:], 1.0 / (SX * SWC),
                    b4[:, oct_:oct_ + 1], mybir.AluOpType.mult,
                    mybir.AluOpType.add)
                sqt = sqp.tile([P, 512], BF, tag="sq")
                if SQ_POOL:
                    nc.gpsimd.tensor_tensor(
                        sqt[:], xrc[:, oct_, ns], xrc[:, oct_, ns],
                        mybir.AluOpType.mult)
                else:
                    nc.scalar.activation(
                        sqt[:], cps[:],
                        mybir.ActivationFunctionType.Square,
                        bias=b4[:, oct_:oct_ + 1], scale=1.0 / (SX * SWC))
                nc.tensor.matmul(vt[:], ones_inv[:], sqt[:],
                                 start=(oct_ == 0), stop=(oct_ == 3))
            return vt

        def emit_ln(n, vt):
            ns = slice(n * 512, (n + 1) * 512)
            nc.scalar.activation(
                stdt[0:1, ns], vt[:],
                mybir.ActivationFunctionType.Ln, bias=eps1[0:1, 0:1])
            nc.scalar.activation(rstd[0:1, ns], stdt[0:1, ns],
                                 mybir.ActivationFunctionType.Exp, scale=-0.5)
            # bf16 rbb: all-bf16 SBUF operands let the xrn tensor_tensor hit
            # the DVE 2x/4x perf mode (rel-err impact ~1e-4)
            rbb = rbbp.tile([P, 512], BF, tag="rbb")
            nc.gpsimd.partition_broadcast(rbb[:], rstd[0:1, ns])
            rbb4 = rbb[:].rearrange("p (o n) -> p o n", o=1).broadcast_to(
                [P, 4, 512])
            nc.vector.tensor_tensor(
                xrn[:, :, ns], xrc[:, :, ns], rbb4,
                mybir.AluOpType.mult)

        def emit_k(n):
            ns = slice(n * 512, (n + 1) * 512)
            for m in range(2):
                kps = wkps.tile([P, 512], F32, tag="wk")
                for kt in range(4):
                    nc.tensor.matmul(
                        kps[:], kw[:, kt, m * P:(m + 1) * P], xrn[:, kt, ns],
                        start=(kt == 0), stop=False)
                # + bias as rank-1 outer product folded into the PSUM chain
                nc.tensor.matmul(
                    kps[:], kbr[0:1, m * P:(m + 1) * P], ones1[0:1, :],
                    start=False, stop=True, skip_group_check=True)
                nc.scalar.activation(
                    kT8[:, m, 0, ns], kps[:],
                    mybir.ActivationFunctionType.Identity, scale=SK)
                # fp8 residual into pair-1: scores DR sums (k_hi + k_lo).T @ q
                # (the broadcast rhs feeds q to both pairs), recovering the
                # fp8 storage rounding of k for free.
                nc.vector.scalar_tensor_tensor(
                    kT8[:, m, 1, ns], kps[:], SK,
                    kT8[:, m, 0, ns],
                    mybir.AluOpType.mult, mybir.AluOpType.subtract)

        def emit_v(n):
            for pt in range(4 * n, 4 * (n + 1)):
                vps = vpps.tile([P, 256], F32, tag="vp")
                for kt in range(4):
                    nc.tensor.matmul(
                        vps[:], xrn[:, kt, pt * P:(pt + 1) * P], vw[:, kt, :],
                        start=(kt == 0), stop=(kt == 3))
                nc.vector.scalar_tensor_tensor(
                    v4[:, pt, :, 1:65],
                    vps[:].rearrange("p (h d) -> p h d", h=4),
                    0.0,
                    vbb[:].rearrange("p (h d) -> p h d", h=4),
                    mybir.AluOpType.add,
                    mybir.AluOpType.add)

        # ---------------- phase 2 emitters ----------------
        state = {}

        def _qb(i):
            ci, h = i // 4, i % 4
            cs = slice(ci * CHUNK, (ci + 1) * CHUNK)
            po = 64 * (h % 2)
            return qT8[po:po + 64, h // 2, cs].rearrange(
                "p (one n) -> p one n", one=1).broadcast_to([64, 2, CHUNK])

        # static per-task exp split balancing ACT vs DVE steady-state load:
        # j0,j1 and tokens [0:YSPL) of j2 on ACT; rest of j2 and j3 on DVE
        # (DVE also carries norm+recip+netT+osb).
        YSPL = int(_os.environ.get("K_YSPL", "452"))

        D1_SPLIT = int(_os.environ.get("K_D1_SPLIT", "0"))
        # d2-task j1 token split: DVE keeps tokens [0:J1TS), ACT takes the
        # tail as an extra small call — lowers the DVE steady-state ceiling
        # (3297 -> ~3060ns/iter) without changing the j alternation order.
        J1TS = int(_os.environ.get("K_J1TS", "0"))

        def emit_exp(sps, ebf, i, j):
            if _exp_on_dve(i, j):
                if J1TS and j == 1:
                    nc.vector._custom_dve(
                        EXP8, out=ebf[:, :, 0:J1TS], in0=sps[:, :, 0:J1TS],
                        s0=EXP8_C[0], s1=EXP8_C[1], imm2=EXP8_C[2])
                    nc.scalar.activation(
                        ebf[:, :, J1TS:CHUNK], sps[:, :, J1TS:CHUNK],
                        mybir.ActivationFunctionType.Exp, scale=SCALE / SK)
                    return
                nc.vector._custom_dve(
                    EXP8, out=ebf[:], in0=sps[:],
                    s0=EXP8_C[0], s1=EXP8_C[1], imm2=EXP8_C[2])
            elif (D1_SPLIT and j == 1 and i % 4 >= EXP_DVE_EXTRA
                  and i >= W):
                # d=1 task: ACT would carry 3 full exps (ceiling 3726ns)
                # while DVE idles ~950ns — split this one token-wise
                y = D1_SPLIT
                nc.scalar.activation(
                    ebf[:, :, 0:y], sps[:, :, 0:y],
                    mybir.ActivationFunctionType.Exp, scale=SCALE / SK)
                nc.vector._custom_dve(
                    EXP8, out=ebf[:, :, y:CHUNK], in0=sps[:, :, y:CHUNK],
                    s0=EXP8_C[0], s1=EXP8_C[1], imm2=EXP8_C[2])
            else:
                nc.scalar.activation(
                    ebf[:], sps[:],
                    mybir.ActivationFunctionType.Exp, scale=SCALE / SK)

        def emit_scores(i, js=(0, 1, 2, 3)):
            h = i % 4
            qb = _qb(i)
            etiles = state.setdefault(i, [])
            if SPS1:
                for j in js:
                    for ti in range(2):
                        t = 2 * j + ti
                        sps = spsp.tile([P, 1, CHUNK], F32, tag="s")
                        nc.tensor.matmul(
                            sps[:, 0, :],
                            kT8[0:64, h, :, t * P:(t + 1) * P],
                            qb, start=True, stop=True, perf_mode=DRM)
                        ebf = esb.tile([P, 1, CHUNK], BF, tag="e")
                        if _exp_on_dve(i, 2 * j + ti):
                            nc.vector._custom_dve(
                                EXP8, out=ebf[:], in0=sps[:],
                                s0=EXP8_C[0], s1=EXP8_C[1], imm2=EXP8_C[2])
                        else:
                            nc.scalar.activation(
                                ebf[:], sps[:],
                                mybir.ActivationFunctionType.Exp, scale=SCALE / SK)
                        etiles.append((ebf, 0))
                return
            for j in js:
                if j == 2 and DUMMY_SC:
                    emit_dummy(DUMMY_SC)
                sps = spsp.tile([P, 2, CHUNK], F32, tag="s")
                po = 64 * (h % 2)
                for ti in range(2):
                    t = 2 * j + ti
                    nc.tensor.matmul(
                        sps[:, ti, :],
                        kT8[po:po + 64, h // 2, :, t * P:(t + 1) * P],
                        qb, start=True, stop=True, perf_mode=DRM)
                ebf = esb.tile([P, 2, CHUNK], BF, tag="e")
                emit_exp(sps, ebf, i, j)
                etiles.append((ebf, 0))
                etiles.append((ebf, 1))

        def emit_scores_early(i):
            """Chunks 0-3 of task i as single-chunk tiles (phase-1 PSUM)."""
            h = i % 4
            qb = _qb(i)
            etiles = state.setdefault(i, [])
            for t in range(4):
                sps = esps.tile([P, 1, CHUNK], F32, tag="es")
                po = 64 * (h % 2)
                nc.tensor.matmul(
                    sps[:, 0, :],
                    kT8[po:po + 64, h // 2, :, t * P:(t + 1) * P],
                    qb, start=True, stop=True, perf_mode=DRM)
                ebf = eesb.tile([P, 1, CHUNK], BF, tag="ee")
                if t % 2 == 1:
                    nc.vector._custom_dve(
                        EXP8, out=ebf[:], in0=sps[:],
                        s0=EXP8_C[0], s1=EXP8_C[1], imm2=EXP8_C[2])
                else:
                    nc.scalar.activation(
                        ebf[:], sps[:],
                        mybir.ActivationFunctionType.Exp, scale=SCALE / SK)
                etiles.append((ebf, 0))

        def emit_dummy(n_mm):
            """PE keep-warm: junk matmuls into a sacrificial PSUM bank so the
            tensor engine's p-state ramp survives element-engine waits."""
            dt_ = dmyps.tile([P, CHUNK], F32, tag="dm")
            rhs = ident[:].rearrange("p (o n) -> p o n", o=1).broadcast_to(
                [P, 4, P])
            for d in range(n_mm):
                nc.tensor.matmul(dt_[:], ident[:], rhs, start=True, stop=True)

        def emit_av_head(i):
            h = i % 4
            etiles = state.pop(i)
            avp = avps.tile([P, 4, 65], F32, tag="av")
            # t-outer: the first matmuls only need etile[0], so av starts
            # while the later exp instructions are still running.
            if AV_TOUTER:
                order = [(t, s) for t in range(8) for s in range(4)]
            else:
                order = [(t, s) for s in range(4) for t in range(8)]
            for t, s in order:
                ebf, sub = etiles[t]
                nc.tensor.matmul(
                    avp[:, s, :],
                    ebf[:, sub, s * P:(s + 1) * P],
                    v4[:, t, h, :],
                    start=(t == 0), stop=(t == 7))
            rec = recp.tile([P, 4, 1], F32, tag="rec")
            nc.vector.reciprocal_approx_fast(out=rec[:], in_=avp[:, :, 0:1])
            net = netp.tile([P, 4, 64], BF, tag="net")
            nc.vector.tensor_tensor(
                net[:], avp[:, :, 1:65], rec[:].broadcast_to([P, 4, 64]),
                mybir.AluOpType.mult)
            state[("net", i)] = net

        def emit_av_tail(i):
            ci, h = i // 4, i % 4
            cs = slice(ci * CHUNK, (ci + 1) * CHUNK)
            net = state.pop(("net", i))
            ntp = ntpp.tile([64, 4, P], BF, tag="nt")
            for s in range(4):
                nc.tensor.transpose(ntp[:, s, :], net[:, s, :], ident[:])
            po = 64 * (h % 2)
            dst = netT[po:po + 64, h // 2, cs].rearrange(
                "p (s n) -> p s n", s=4)
            if NTCOPY_ALT and i % 2 == 0:
                nc.scalar.activation(dst, ntp[:],
                                     mybir.ActivationFunctionType.Identity)
            else:
                nc.vector.tensor_copy(dst, ntp[:])

        def emit_proj_oct(ci, oct_, eng="act", pool=None, single=False):
            cs = slice(ci * CHUNK, (ci + 1) * CHUNK)
            pps = (pool or prps).tile([P, CHUNK], F32, tag="pr")
            for kt in range(2):
                nc.tensor.matmul(
                    pps[:], pw[:, kt, oct_ * P:(oct_ + 1) * P],
                    netT[:, kt, cs],
                    start=(kt == 0), stop=(kt == 1))
            # per-chunk [P, 4, CHUNK] staging: the 4 octs are contiguous row
            # blocks of out_d, so one merged DMA replaces 4 small ones
            # (saves ~0.7us of serial HWDGE per chunk at the drain tail).
            if oct_ == 0:
                osb4 = osbp.tile([P, 4, CHUNK], BF, tag="osb")
                state[("osb4", ci)] = osb4
            osb = state[("osb4", ci)]
            # osb on ACT in steady state; the drain loop alternates engines
            # to shorten the serial tail.
            if eng == "dve":
                nc.vector.tensor_copy(osb[:, oct_, :], pps[:])
            else:
                nc.scalar.activation(
                    osb[:, oct_, :], pps[:],
                    mybir.ActivationFunctionType.Identity)
            if single and oct_ >= 2:
                # drain mode: per-oct DMAs so the final transfer is half
                # size (oct3 may still pair with oct2 if oct2 was emitted
                # by the main loop's pair path)
                if oct_ == 2:
                    state[("s2", ci)] = True
                    nc.sync.dma_start(
                        out_d[2 * P:3 * P, cs].rearrange(
                            "(o p) n -> p o n", o=1),
                        osb[:, 2:3, :])
                else:
                    state.pop(("osb4", ci))
                    if state.pop(("s2", ci), False):
                        nc.sync.dma_start(
                            out_d[3 * P:4 * P, cs].rearrange(
                                "(o p) n -> p o n", o=1),
                            osb[:, 3:4, :])
                    else:
                        nc.sync.dma_start(
                            out_d[256:512, cs].rearrange(
                                "(o p) n -> p o n", o=2),
                            osb[:, 2:4, :])
            elif oct_ in (1, 3):
                # fire the output DMA per oct-pair so the second pair's
                # evacuation overlaps the first pair's transfer
                if oct_ == 3:
                    state.pop(("osb4", ci))
                half = oct_ // 2
                nc.sync.dma_start(
                    out_d[half * 256:(half + 1) * 256, cs].rearrange(
                        "(o p) n -> p o n", o=2),
                    osb[:, 2 * half:2 * half + 2, :])

        # ---------------- emission schedule ----------------
        W = int(_os.environ.get("K_W", "7"))
        # PE ramp pre-warm: junk matmuls (gated only on the warm memset,
        # ready ~0.4us) keep the p-state ramp running during the input DMA
        # stream so q runs at full rate from its first instruction (~5.5us).
        WARM_VT = int(_os.environ.get("K_WARM_VT", "22"))
        for _wi in range(WARM_VT):
            wvt = vtps.tile([1, 512], F32, tag="vt")
            nc.tensor.matmul(
                wvt[0:1, 0:256], warm[:, 0:1], warm[:],
                start=True, stop=True)
        if DUMMY_WARM:
            emit_dummy(DUMMY_WARM)  # warm the PE ramp during DMA wait
        nq_pre = int(_os.environ.get("K_NQ_PRE", "8"))
        for ci in range(8):
            for m in range(2):
                if ci < nq_pre:
                    emit_q(ci, m)
        PH1_ORDER = int(_os.environ.get("K_PH1_ORDER", "0"))
        if PH1_ORDER == 2:
            # k0/v0/early-scores interleaved BETWEEN conv1's oct chains:
            # early exps (16us of elementwise) start ~27us instead of ~37,
            # filling the ACT/DVE idle window, while conv1's octs keep PE
            # fed between the small emissions (no esps-pool stall).
            vt0 = emit_conv(0)
            emit_ln(0, vt0)
            eq = list(range(W))

            def _ao(o):
                if o == 1:
                    emit_k(0)
                    emit_v(0)
                elif o == 2:
                    for _ in range(4):
                        if eq:
                            emit_scores_early(eq.pop(0))
                elif o == 3:
                    for _ in range(3):
                        if eq:
                            emit_scores_early(eq.pop(0))
            vt1 = emit_conv(1, after_oct=_ao)
            while eq:
                emit_scores_early(eq.pop(0))
            emit_ln(1, vt1)
            emit_k(1)
            emit_v(1)
        elif PH1_ORDER == 1:
            # k0 + early-scores BEFORE conv1: their exps (16us of
            # elementwise) start ~26us instead of ~37, filling the ACT/DVE
            # idle window while PE grinds conv1 (at the cost of k1 landing
            # ~4us later).
            vt0 = emit_conv(0)
            emit_ln(0, vt0)
            emit_k(0)
            emit_v(0)
            for i in range(W):
                emit_scores_early(i)
            vt1 = emit_conv(1)
            emit_ln(1, vt1)
            emit_k(1)
            emit_v(1)
        else:
            vt0 = emit_conv(0)
            emit_ln(0, vt0)
            for ci in range(nq_pre, 8):
                for m in range(2):
                    emit_q(ci, m)
            vt1 = emit_conv(1)
            emit_k(0)
            emit_v(0)
            emit_ln(1, vt1)
            # early half-scores (kv chunks 0-3) for the first W tasks: their
            # exps keep ACT/DVE busy while PE finishes k1/v1.
            if W:
                emit_scores_early(0)
                emit_scores_early(1)
            emit_k(1)
            for i in range(2, W):
                if i == 4:
                    emit_v(1)
                emit_scores_early(i)
            if W <= 4:
                emit_v(1)
        ph1.close()

        ph2 = ExitStack()
        spsp = ph2.enter_context(tc.tile_pool(
            name="sps",
            bufs=(5 if SPS1 else int(_os.environ.get("K_SPS_BUFS", "2"))),
            space="PSUM"))
        avps = ph2.enter_context(tc.tile_pool(
            name="avp", bufs=int(_os.environ.get("K_AVP_BUFS", "2")),
            space="PSUM"))
        ntpp = ph2.enter_context(tc.tile_pool(
            name="ntp", bufs=int(_os.environ.get("K_NTP_BUFS", "1")),
            space="PSUM"))
        prps = ph2.enter_context(tc.tile_pool(
            name="prj", bufs=int(_os.environ.get("K_PRJ_BUFS", "1")),
            space="PSUM"))
        NT = 32
        projq = []
        for i in range(NT + 2):
            # first scores half feeds the exp engines immediately ...
            import os as _o
            SPLITV = int(_o.environ.get("K_SPLITV", "2"))
            # AV_FIRST=0: av of the previous task emitted before this task's
            # first score half (swapped engine-FIFO order experiment)
            if AV_FIRST == 0 and 1 <= i <= NT:
                emit_av_head(i - 1)
            if i < NT:
                if i < W:
                    emit_scores(i, (2,))
                elif SPLITV == 1:
                    emit_scores(i, (0,))
                else:
                    emit_scores(i, (0, 1))
            # ... av of the previous task fills PE while exp drains the
            # score buffers the second half needs
            if AV_FIRST != 0 and 1 <= i <= NT:
                if DUMMY_AV:
                    emit_dummy(DUMMY_AV)
                emit_av_head(i - 1)
            # On d=1 tasks (i%4==3: only the j3 exp runs on DVE) the DVE
            # queue would stall at the not-yet-ready exp; emit the ready
            # netT/osb work of older tasks first so the FIFO stays fed.
            d1 = (i % 4 == 3) and i >= W and int(
                _o.environ.get("K_D1_REORD", "0"))
            if d1:
                if 2 <= i <= NT + 1:
                    emit_av_tail(i - 2)
                    if (i - 2) % 4 == 3:
                        projq += [((i - 2) // 4, o) for o in range(4)]
                if projq:
                    emit_proj_oct(*projq.pop(0))
            if i < NT:
                if i < W:
                    emit_scores(i, (3,))
                else:
                    emit_scores(i, (1, 2, 3) if SPLITV == 1 else (2, 3))
            if not d1:
                if 2 <= i <= NT + 1:
                    emit_av_tail(i - 2)
                    if (i - 2) % 4 == 3:
                        projq += [((i - 2) // 4, o) for o in range(4)]
                if projq:
                    # osb emitted at the END of i%4==2 iterations lands in
                    # DVE's FIFO just before the d=1 task's recip, filling
                    # the ~950ns DVE stall observed there (ACT is busy with
                    # 3 exp quarters on those tasks).
                    o2d = int(_o.environ.get("K_OSB2DVE", "0"))
                    emit_proj_oct(*projq.pop(0),
                                  eng=("dve" if (o2d and i % 4 == 2 and
                                                 i >= W) else "act"))
        # drain: scores/av/transpose PSUM pools are done — release their
        # banks and pipeline the remaining proj-octs through a wide pool
        # with alternating evacuation engines.
        ph2.close()
        if projq:
            with tc.tile_pool(name="prj2", bufs=4, space="PSUM") as prps2:
                drain_i = 0
                while projq:
                    emit_proj_oct(*projq.pop(0),
                                  eng=("act" if drain_i % 2 else "dve"),
                                  pool=prps2,
                                  single=bool(int(_os.environ.get(
                                      "K_DRAIN_SINGLE", "0"))))
                    drain_i += 1

        if DEBUG:
            with tc.tile_pool(name="dbg", bufs=1) as dp:
                def dump(dram, sb_ap, nparts):
                    total = 1
                    for s in sb_ap.shape[1:]:
                        total *= s
                    pat = {2: None, 3: "p a b -> p (a b)",
                           4: "p a b c -> p (a b c)"}[len(sb_ap.shape)]
                    flat = sb_ap.rearrange(pat) if pat else sb_ap
                    dfl = dram.rearrange(pat) if pat else dram
                    for off in range(0, total, 512):
                        w_ = min(512, total - off)
                        tt = dp.tile([nparts, 512], F32, tag="dbgt")
                        nc.vector.tensor_copy(tt[:, :w_], flat[:, off:off + w_])
                        nc.sync.dma_start(dfl[:, off:off + w_], tt[:, :w_])
                dump(dbg["xrn"], xrn[:], P)
                dump(dbg["qT8"], qT8[:], P)
                dump(dbg["kT8"], kT8[:], P)
                dump(dbg["v4"], v4[:], P)
                dump(dbg["netT"], netT[:], P)

    nc.finalize()
    return nc


_NC_CACHE = {}


def _get_nc():
    if "nc" not in _NC_CACHE:
        _NC_CACHE["nc"] = build_nc()
    return _NC_CACHE["nc"]


def _token_perm():
    """tokp = t*1024 + p (t = 2di+dj, p = 32i+j) -> original token index."""
    di = np.array([0, 0, 1, 1])
    dj = np.array([0, 1, 0, 1])
    i, j = np.meshgrid(np.arange(32), np.arange(32), indexing="ij")
    perm = np.empty(4096, np.int64)
    for t in range(4):
        r = (2 * i + di[t]) * 64 + (2 * j + dj[t])
        perm[t * 1024:(t + 1) * 1024] = r.reshape(-1)
    return perm


TOKPERM = _token_perm()


def _prep_core_inputs(x, q_w, kv_w, proj_w, sr_w, sr_b, ln_w, ln_b):
    # conv weights: W4[t, ic, oc] = sr_w[oc, ic, di, dj], t = di*2+dj,
    # centered over oc so LN mean-subtraction is free.
    W4 = np.ascontiguousarray(sr_w.transpose(2, 3, 1, 0)).reshape(4, DIM, DIM)
    W4 = W4 - W4.mean(axis=2, keepdims=True)
    # wc[k = c*4 + t][p][oc] = W4[t, c*128+p, oc]
    wc = np.ascontiguousarray(
        W4.reshape(4, 4, P, DIM).transpose(1, 0, 2, 3)).reshape(16, P, DIM)
    wcs = wc * SWC
    wch = wcs.astype(NF8)
    wcl = (wcs - wch.astype(np.float32)).astype(NF8)
    wc2 = np.concatenate([wch, wcl], axis=0)               # (32, P, DIM)
    kv_w_f = ln_w[:, None] * kv_w
    kv_bias = ln_b @ kv_w
    b4 = (sr_b - sr_b.mean()).reshape(4, P).T.astype(np.float32)
    b4 = np.ascontiguousarray(b4)

    in_maps = []
    for core in range(8):
        b, g = core // 2, core % 2
        xT = np.ascontiguousarray(x[b].T[:, TOKPERM])      # (512, 4096) permuted
        xs = xT.reshape(4, P, 4, NKV) * SX                 # (c, P, t, pos)
        xh8 = xs.astype(NF8)
        xl8 = (xs - xh8.astype(np.float32)).astype(NF8)
        xh8t = np.ascontiguousarray(xh8.transpose(2, 0, 1, 3))  # (t, c, P, pos)
        xl8p = xl8.transpose(1, 0, 2, 3)                   # (P, c, t, pos)
        xl8h = np.ascontiguousarray(np.stack(
            [xl8p[:, :, :, 0:512].reshape(P, 16, 512),
             xl8p[:, :, :, 512:1024].reshape(P, 16, 512)], axis=0))
        qsl = q_w[:, g * 256:(g + 1) * 256]
        # qw8[p, u, cc, col] = q_w[(2u+cc)*128 + p, g*256+col]
        qarr = np.ascontiguousarray(
            qsl.reshape(2, 2, P, 256).transpose(2, 0, 1, 3)) * SQW
        qw8 = qarr.astype(NF8)
        ql8 = (qarr - qw8.astype(np.float32)).astype(NF8)
        qwl = np.ascontiguousarray(
            np.stack([qw8, ql8], axis=1))                  # (P, 2, 2, 2, 256)
        kw = np.ascontiguousarray(
            kv_w_f[:, g * 256:(g + 1) * 256]).reshape(4, P, 256).astype(NBF)
        vw = np.ascontiguousarray(
            kv_w_f[:, DIM + g * 256:DIM + (g + 1) * 256]
        ).reshape(4, P, 256).astype(NBF)
        kvw = np.concatenate([kw, vw], axis=0)             # (8, P, 256)
        pwv = np.ascontiguousarray(
            proj_w[g * 256:(g + 1) * 256, :]).reshape(2, P, DIM).astype(NBF)
        kbv = kv_bias[g * 256:(g + 1) * 256].astype(NBF)
        vbv = kv_bias[DIM + g * 256:DIM + (g + 1) * 256].astype(NBF)
        kvb = np.concatenate([kbv, vbv]).reshape(1, 512)
        in_maps.append({
            "xh8": xh8t, "xl8": xl8h, "wc2": wc2, "qwl": qwl,
            "kvw": kvw, "pw": pwv, "b4": b4, "kvb": kvb,
        })
    return in_maps


def kernel(x, q_w, kv_w, proj_w, proj_b, sr_w, sr_b, ln_w, ln_b, H, W,
           _return_perf=False):
    x = np.asarray(x, dtype=np.float32)
    q_w = np.asarray(q_w, dtype=np.float32)
    kv_w = np.asarray(kv_w, dtype=np.float32)
    proj_w = np.asarray(proj_w, dtype=np.float32)
    proj_b = np.asarray(proj_b, dtype=np.float32)
    sr_w = np.asarray(sr_w, dtype=np.float32)
    sr_b = np.asarray(sr_b, dtype=np.float32)
    ln_w = np.asarray(ln_w, dtype=np.float32)
    ln_b = np.asarray(ln_b, dtype=np.float32)

    in_maps = _prep_core_inputs(x, q_w, kv_w, proj_w, sr_w, sr_b, ln_w, ln_b)
    nc = _get_nc()
    res = run_bass_kernel_spmd(nc, in_maps, core_ids=list(range(8)),
                               trace=_return_perf)
    out = np.empty((BS, N, DIM), dtype=np.float32)
    for b in range(BS):
        partial = (res.results[2 * b]["out"].astype(np.float32)
                   + res.results[2 * b + 1]["out"].astype(np.float32))
        out[b][TOKPERM, :] = partial.T
        out[b] += proj_b[None, :]
    if _return_perf:
        return out, res
    return out

